# revision 4
# baseline (speedup 1.0000x reference)
"""AttentiveConv3d (sparse_attention) Trainium2 kernel — self-contained.

kernel(**inputs) takes the FULL inputs
    x     [2, 128, 16, 28, 28] f32
    q     [2, 1, 64] f32
    W_out [128, 128] f32
    b_out [128] f32
and returns the FULL output [2, 128, 16, 28, 28] f32.

Sharding: data-parallel over (batch, T-chunks): 8 cores, core i handles
batch i//4, output frames 4*(i%4) .. 4*(i%4)+3, with a 1-frame halo supplied
by host-side padding/slicing (no device collectives needed).

Math (equivalent to the reference; softmax computed without max-subtraction,
valid because |logits| < ~0.2 for this operator's scaling):
    z   = qmask^T @ xp        (per padded location; both heads)
    E   = exp(z);  F = E * xp
    num = Box3x3x3(F); d = Box3x3x3(E)    (separable box filters)
    y   = W_out @ (num / d) + b_out

v2: fp16 fields end-to-end (f32 PSUM accumulation), host-padded input so
exp(z)=1 / F=0 pads come for free, 36-row denominator pack (dy baked into
the pack, dx via 3 accumulating matmuls), DMA loads from t=0 on the SP
queue, stores on the ACT queue, pack DMAs on the Pool queue.
"""
from contextlib import ExitStack

import numpy as np

import concourse.bass as bass
import concourse.tile as tile
from concourse import bacc, mybir
from concourse import bass_utils

F32 = mybir.dt.float32
F32R = mybir.dt.float32r
F16 = mybir.dt.float16
AF = mybir.ActivationFunctionType

C = 128
TIN, TOUT = 6, 4
HP, WP = 30, 30
HO, WO = 28, 28
NF = HP * WP        # 900
NOF = HO * WO       # 784


def _build_nc(num_devices=8, reps=1, n_warm=6,
              h_pool_frames=(), f_pool_frames=()):
    nc = bacc.Bacc("TRN2", target_bir_lowering=False, debug=False,
                   num_devices=num_devices)
    d_xp = nc.dram_tensor("xp", [C, TIN, NF], F16, kind="ExternalInput").ap()
    d_cst = nc.dram_tensor("cst", [C, 384], F16, kind="ExternalInput").ap()
    d_bias = nc.dram_tensor("bias", [C, 1], F32, kind="ExternalInput").ap()
    d_selr = nc.dram_tensor("selr", [8, TOUT * C], F32R, kind="ExternalInput").ap()
    d_sel36 = nc.dram_tensor("sel36", [36, 8], F16, kind="ExternalInput").ap()
    d_y = nc.dram_tensor("y", [C, TOUT, NOF], F16, kind="ExternalOutput").ap()

    with tile.TileContext(nc) as tc:
        with ExitStack() as ctx:
            consts = ctx.enter_context(tc.tile_pool(name="consts", bufs=1))
            sb_x = ctx.enter_context(tc.tile_pool(name="sb_x", bufs=1))
            sb_e = ctx.enter_context(tc.tile_pool(name="sb_e", bufs=1))
            sb_f = ctx.enter_context(tc.tile_pool(name="sb_f", bufs=1))
            sb_s = ctx.enter_context(tc.tile_pool(name="sb_s", bufs=3))
            sb_tmp = ctx.enter_context(tc.tile_pool(name="sb_tmp", bufs=3))
            sb_m = ctx.enter_context(tc.tile_pool(name="sb_m", bufs=2))
            sb_y = ctx.enter_context(tc.tile_pool(name="sb_y", bufs=3))
            sb_ep = ctx.enter_context(tc.tile_pool(name="sb_ep", bufs=1))
            ps_big = ctx.enter_context(tc.tile_pool(name="ps_big", bufs=3, space="PSUM"))
            ps_small = ctx.enter_context(tc.tile_pool(name="ps_small", bufs=2, space="PSUM"))

            # Warm-up: ramp the PE p-state on a memset dummy (no DMA dep).
            wrm_sb = consts.tile([C, 512], F16)
            nc.gpsimd.memset(wrm_sb[:], 0.0)
            for i in range(n_warm):
                wrm = ps_small.tile([C, 512], F32, tag="small", name=f"warm{i}")
                nc.tensor.matmul(wrm[:], wrm_sb[:, 0:128], wrm_sb[:],
                                 start=True, stop=True)

            # Const loads on the Pool queue (cheap SEQ) so SP is free for x.
            cst_t = consts.tile([C, 384], F16)
            nc.gpsimd.dma_start(out=cst_t[:], in_=d_cst[:])
            bias_t = consts.tile([C, 1], F32)
            nc.gpsimd.dma_start(out=bias_t[:], in_=d_bias[:])
            selr_t = consts.tile([8, TOUT * C], F32R)
            nc.gpsimd.dma_start(out=selr_t[:], in_=d_selr[:])
            sel36_t = consts.tile([36, 8], F16)
            nc.gpsimd.dma_start(out=sel36_t[:], in_=d_sel36[:])
            qm = cst_t[:, 0:128]
            idm = cst_t[:, 128:256]
            wt = cst_t[:, 256:384]
            bias = bias_t[:]
            selr = selr_t[:].rearrange("p (t c) -> p t c", t=TOUT)

            for _ in range(reps):
                _body(tc, nc, d_xp, d_y, qm, idm, wt, bias, sel36_t, selr,
                      sb_x, sb_e, sb_f, sb_s, sb_tmp, sb_m, sb_y, sb_ep,
                      ps_big, ps_small, h_pool_frames, f_pool_frames)
    nc.compile()
    return nc


def _body(tc, nc, d_xp, d_y, qm, idm, wt, bias, sel36_t, selr,
          sb_x, sb_e, sb_f, sb_s, sb_tmp, sb_m, sb_y, sb_ep,
          ps_big, ps_small, h_pool_frames, f_pool_frames):
    # ---- phase A: per input frame: load, z matmul, E = exp(z), F = E*xp,
    # and the denominator pack rows (dy baked in).
    e128 = sb_e.tile([C, TIN * NF], F16, tag="e128")
    ep36 = sb_ep.tile([36, NF], F16, tag="ep36")
    f_tiles = []
    for f in range(TIN):
        xtt = sb_x.tile([C, NF], F16, tag=f"x{f}", name=f"xt{f}")
        nc.sync.dma_start(out=xtt[:], in_=d_xp[:, f])
        xt = xtt[:]

        zp = ps_big.tile([C, 1024], F32, tag="big", name=f"zp{f}")
        nc.tensor.matmul(zp[:, 0:450], qm, xt[:, 0:450], start=True, stop=True)
        nc.tensor.matmul(zp[:, 512:962], qm, xt[:, 450:900], start=True, stop=True)

        # exp over the full padded frame: z=0 at pads -> E=1 there for free
        ef = e128[:, f * NF:(f + 1) * NF].rearrange("p (b k) -> p b k", b=2)
        zv = zp[:].rearrange("p (b k) -> p b k", b=2)[:, :, 0:450]
        nc.scalar.activation(ef, zv, AF.Exp)

        # F = E * xp over the full frame: xp pads are 0 -> F=0 pads for free
        ft = sb_f.tile([C, NF], F16, tag=f"f{f}", name=f"ft{f}")
        e_flat = e128[:, f * NF:(f + 1) * NF]
        feng = nc.gpsimd if f in f_pool_frames else nc.vector
        feng.tensor_mul(ft[:], e_flat, xt)
        f_tiles.append(ft)

        # denominator pack: ep36[12*dy + 6h + f, i] = E_h[f][30*dy + i]
        ebase = e128[:]
        epbase = ep36[:]
        for h in range(2):
            src = bass.AP(tensor=ebase.tensor,
                          offset=ebase.offset + h * ebase.ap[0][0] + f * NF,
                          ap=[[ebase.ap[0][0], 1], [30, 3], [1, 840]])
            dst = bass.AP(tensor=epbase.tensor,
                          offset=epbase.offset + (6 * h + f) * epbase.ap[0][0],
                          ap=[[epbase.ap[0][0] * 12, 3], [1, 840]])
            nc.gpsimd.dma_start(out=dst, in_=src)

    # ---- denominator: 3 dx-shift matmuls over the 36-row pack, recip ----
    epbase = ep36[:]
    r8f = sb_ep.tile([8, NOF], F32, tag="r8f")
    for ch in range(2):
        ylo = 14 * ch
        d8p = ps_small.tile([8, 392], F32, tag="small", name=f"d8p{ch}")
        for dx in range(3):
            rhs = bass.AP(tensor=epbase.tensor,
                          offset=epbase.offset + 30 * ylo + dx,
                          ap=[[epbase.ap[0][0], 36], [30, 14], [1, WO]])
            nc.tensor.matmul(d8p[:], sel36_t[:], rhs,
                             start=(dx == 0), stop=(dx == 2))
        nc.vector.reciprocal_approx_fast(r8f[:, ch * 392:ch * 392 + 392], d8p[:])
    r8t = sb_ep.tile([8, NOF], F32R, tag="r8")
    nc.scalar.copy(r8t[:], r8f[:])
    r8 = r8t[:]

    # ---- per output frame: T-pass (PE), S evac (ACT), W+H (DVE/Pool),
    # r-mul + projection + bias evac + store
    for t in range(TOUT):
        ftp = ps_big.tile([C, 1024], F32, tag="big", name=f"ftp{t}")
        for half in range(2):
            lo = half * 512
            slo = half * 450
            for dt in range(3):
                nc.tensor.matmul(ftp[:, lo:lo + 450], idm,
                                 f_tiles[t + dt][:, slo:slo + 450],
                                 start=(dt == 0), stop=(dt == 2))
        st = sb_s.tile([C, NF], F16, tag="s", name=f"st{t}")
        sv2 = st[:].rearrange("p (b k) -> p b k", b=2)
        fv2 = ftp[:].rearrange("p (b k) -> p b k", b=2)[:, :, 0:450]
        nc.scalar.activation(sv2, fv2, AF.Copy)

        sv = st[:].rearrange("p (y x) -> p y x", y=HP)
        w1 = sb_tmp.tile([C, HP, WO], F16, tag="w1", name=f"w1_{t}")
        nc.vector.tensor_add(w1[:], sv[:, :, 0:28], sv[:, :, 1:29])
        w2 = sb_tmp.tile([C, HP, WO], F16, tag="w2", name=f"w2_{t}")
        nc.vector.tensor_add(w2[:], w1[:], sv[:, :, 2:30])

        heng = nc.gpsimd if t in h_pool_frames else nc.vector
        h1 = sb_tmp.tile([C, HO, WO], F16, tag="h1", name=f"h1_{t}")
        heng.tensor_add(h1[:], w2[:, 0:28, :], w2[:, 1:29, :])
        numt = sb_tmp.tile([C, HO, WO], F16, tag="numt", name=f"numt{t}")
        heng.tensor_add(numt[:], h1[:], w2[:, 2:30, :])
        nv = numt[:].rearrange("p y x -> p (y x)")

        mt = sb_m.tile([C, NOF], F16, tag="m", name=f"mt{t}")
        yt = sb_y.tile([C, NOF], F16, tag="y", name=f"yt{t}")
        for ch in range(2):
            rp = ps_small.tile([C, 392], F32, tag="small", name=f"rp{t}_{ch}")
            nc.tensor.matmul(rp[:], selr[:, t, :], r8[:, ch * 392:ch * 392 + 392],
                             start=True, stop=True)
            nc.vector.tensor_mul(mt[:, ch * 392:ch * 392 + 392],
                                 nv[:, ch * 392:ch * 392 + 392], rp[:])
            yp = ps_small.tile([C, 392], F32, tag="small", name=f"yp{t}_{ch}")
            nc.tensor.matmul(yp[:], wt, mt[:, ch * 392:ch * 392 + 392],
                             start=True, stop=True)
            nc.scalar.activation(yt[:, ch * 392:ch * 392 + 392], yp[:],
                                 AF.Identity, bias=bias, scale=1.0)
        nc.scalar.dma_start(out=d_y[:, t], in_=yt[:])


# ---------------------------------------------------------------------------
# Host side
# ---------------------------------------------------------------------------

def _host_prep(x, q, W_out, b_out):
    B, C_, T, H, W = x.shape
    heads, hs = 2, 64
    xpad = np.zeros((B, C_, T + 2, HP, WP), np.float16)
    xpad[:, :, 1:T + 1, 1:H + 1, 1:W + 1] = np.asarray(x, np.float32)

    cidx = np.arange(C_)
    qfull = (np.asarray(q, np.float32)[cidx % heads, 0, cidx // heads] / hs)
    qm = np.zeros((C_, C_), np.float32)
    for m in range(C_):
        qm[:, m] = np.where(cidx % heads == m % heads, qfull, 0.0)
    cst = np.zeros((C_, 384), np.float16)
    cst[:, 0:128] = qm
    cst[:, 128:256] = np.eye(C_)
    cst[:, 256:384] = np.asarray(W_out, np.float32).T
    bias = np.asarray(b_out, np.float32).reshape(C_, 1)

    # sel36[12*dy + 6h + t, 4h + tp] = 1 iff 0 <= t - tp <= 2
    sel36 = np.zeros((36, 8), np.float16)
    for dy in range(3):
        for h in range(2):
            for t in range(TIN):
                for tp in range(TOUT):
                    if 0 <= t - tp <= 2:
                        sel36[12 * dy + 6 * h + t, 4 * h + tp] = 1.0
    # selr[4h + tp, tp, c] = 1 iff c % heads == h   (r-broadcast select)
    selr = np.zeros((8, TOUT, C_), np.float32)
    for tp in range(TOUT):
        selr[4 * (cidx % heads) + tp, tp, cidx] = 1.0

    shared = {"cst": cst, "bias": bias, "sel36": sel36,
              "selr": selr.reshape(8, TOUT * C_)}
    in_maps = []
    for core in range(8):
        b, t0 = core // 4, (core % 4) * 4
        xp = np.ascontiguousarray(
            xpad[b, :, t0:t0 + TIN].reshape(C_, TIN, NF))
        in_maps.append({"xp": xp, **shared})
    return in_maps


_NC_CACHE = {}


def _get_nc(reps=1):
    if reps not in _NC_CACHE:
        _NC_CACHE[reps] = _build_nc(reps=reps)
    return _NC_CACHE[reps]


def kernel(x, q, W_out, b_out):
    x = np.asarray(x, np.float32)
    in_maps = _host_prep(x, q, W_out, b_out)
    nc = _get_nc()
    res = bass_utils.run_bass_kernel_spmd(nc, in_maps, list(range(8)))
    y = np.zeros((2, 128, 16, 28, 28), np.float32)
    for core in range(8):
        b, t0 = core // 4, (core % 4) * 4
        y[b, :, t0:t0 + TOUT] = np.asarray(
            res.results[core]["y"], np.float32).reshape(C, TOUT, HO, WO)
    return y
